# revision 27
# baseline (speedup 1.0000x reference)
"""Trainium2 Bass kernel v2 for nn_EndpointRegressor (2x TransformerConv GNN +
AttentionalAggregation) distributed over 8 NeuronCores.

Sharding: edges partitioned by destination node range (6272 nodes/core, with a
per-core node->window permutation that balances per-window edge counts); each
core owns its dst nodes exclusively so segment softmax needs no cross-core
reduction. Per layer each core computes k|v for its own nodes (bf16), the kv
table is AllGather-ed, and each core dma_gathers kv rows for its edge shard.

Key structure vs v1: everything bf16 on the edge path (single-pass matmuls +
FWL weight loads), the edge-feature term is factored out of the per-chunk
matmuls (rank-5 "B-trick": alpha picks it up via 20 extra q-columns, the
output picks it up via 24 extra scatter columns fixed up per window), the
one-hot transpose runs on the vector engine via a host-side 32-block preswap,
layer-0 folds W_in into the weights (no h0 materialization), h/hT live in
SBUF between layers, and the graph-pool gate is fused into layer-1 window
tails.
"""
import math
import numpy as np
import ml_dtypes

import concourse.bass as bass
import concourse.bacc as bacc
import concourse.mybir as mybir
import concourse.tile as tile
from concourse._compat import get_trn_type
from concourse.bass_utils import run_bass_kernel_spmd
from concourse.library_config import mlp

# ---- problem constants ----
N, E, G = 50000, 500000, 32
H, D = 4, 40
HID = H * D            # 160
JK = 2 * HID           # 320
NCORES = 8
NSHARD = 6272          # 49*128
NPAD = NCORES * NSHARD
WIN = NSHARD // 128    # 49
SPLIT = NPAD // 2      # 25088
GS = 1024
ROW = 384              # kv row: [k160 | v160 | pad64]
INVSQD = 1.0 / math.sqrt(float(D))

f32 = mybir.dt.float32
bf16 = mybir.dt.bfloat16
i16 = mybir.dt.int16
BF = ml_dtypes.bfloat16
AF = mybir.ActivationFunctionType
OP = mybir.AluOpType


def _wrap16(ix):
    return np.tile(ix.reshape(-1, 16).T, (8, 1))


# ============================ host preprocessing ============================

def _pack_core(degL, degH):
    """Assign NSHARD nodes to 49 windows x 128 slots s.t. per-window L/H edge
    sums fit chunk capacities. Returns (win_of, slot_of, needL, needH) where
    needX[w] = chunks needed. Heap-based LPT greedy in pure python."""
    import heapq
    totL, totH = int(degL.sum()), int(degH.sum())
    kL = max(0, -(-(totL - WIN * 640) // 128)) + 2
    kH = max(0, -(-(totH - WIN * 640) // 128)) + 2
    dL = degL.tolist()
    dH = degH.tolist()
    order = sorted(range(NSHARD), key=lambda n: -(dL[n] + dH[n]))
    for attempt in range(8):
        capL = [768 if w < min(WIN, kL) else 640 for w in range(WIN)]
        capH = [768 if w < min(WIN, kH) else 640 for w in range(WIN)]
        remL, remH = list(capL), list(capH)
        cnt = [0] * WIN
        win_of = np.full(NSHARD, -1, np.int64)
        heap = [(-(remL[w] + remH[w]), w) for w in range(WIN)]
        heapq.heapify(heap)
        ok = True
        for node in order:
            a, b = dL[node], dH[node]
            popped = []
            w = -1
            while heap:
                item = heapq.heappop(heap)
                ww = item[1]
                if cnt[ww] < 128 and remL[ww] >= a and remH[ww] >= b:
                    w = ww
                    break
                popped.append(item)
            for it in popped:
                heapq.heappush(heap, it)
            if w < 0:
                ok = False
                break
            win_of[node] = w
            remL[w] -= a
            remH[w] -= b
            cnt[w] += 1
            if cnt[w] < 128:
                heapq.heappush(heap, (-(remL[w] + remH[w]), w))
        if ok:
            needL = np.maximum(1, [-(-(capL[w] - remL[w]) // 128)
                                   for w in range(WIN)])
            needH = np.maximum(1, [-(-(capH[w] - remH[w]) // 128)
                                   for w in range(WIN)])
            # slot within window, in node order
            ordn = np.lexsort((np.arange(NSHARD), win_of))
            slot_of = np.zeros(NSHARD, np.int64)
            slot_of[ordn] = (np.arange(NSHARD)
                             - np.searchsorted(win_of[ordn], win_of[ordn]))
            return win_of, slot_of, needL, needH
        kL += 2
        kH += 2
    raise RuntimeError("packing failed")


def _preprocess(x, edge_index, edge_attr, batch):
    src = np.asarray(edge_index[0], dtype=np.int64)
    dst = np.asarray(edge_index[1], dtype=np.int64)
    ea = np.asarray(edge_attr, dtype=np.float32)
    x = np.asarray(x, dtype=np.float32)
    batch = np.asarray(batch, dtype=np.int64)

    dst_core = dst // NSHARD
    isL = (src // NSHARD) < 4

    cores = []
    needLs, needHs = [], []
    for r in range(NCORES):
        em = dst_core == r
        e_src, e_local, e_ea, e_isL = src[em], dst[em] - r * NSHARD, ea[em], isL[em]
        degL = np.bincount(e_local[e_isL], minlength=NSHARD)
        degH = np.bincount(e_local[~e_isL], minlength=NSHARD)
        win_of, slot_of, needL, needH = _pack_core(degL, degH)
        cores.append((e_src, e_local, e_ea, e_isL, win_of, slot_of))
        needLs.append(needL)
        needHs.append(needH)
    cnL = np.max(needLs, axis=0)
    cnH = np.max(needHs, axis=0)
    oL = np.concatenate([[0], np.cumsum(cnL)])
    oH = np.concatenate([[0], np.cumsum(cnH)])
    TCL, TCH = int(oL[-1]), int(oH[-1])
    NGL = (TCL * 128 + GS - 1) // GS
    NGH = (TCH * 128 + GS - 1) // GS

    pos_of = np.zeros(NPAD, np.int64)
    for r in range(NCORES):
        (_, _, _, _, win_of, slot_of) = cores[r]
        pos_of[r * NSHARD : (r + 1) * NSHARD] = win_of * 128 + slot_of
    table_row = (np.arange(NPAD) // NSHARD) * NSHARD + pos_of

    per_core = []
    for r in range(NCORES):
        (e_src, e_local, e_ea, e_isL, win_of, slot_of) = cores[r]
        ewin = win_of[e_local]
        eslot = slot_of[e_local]

        idxL = np.zeros(NGL * GS, np.int64)
        idxH = np.zeros(NGH * GS, np.int64)
        dstrelL = np.full((128, TCL), -1.0, np.float32)
        dstrelH = np.full((128, TCH), -1.0, np.float32)
        eaE6L = np.zeros((128, TCL, 6), np.float32)
        eaE6H = np.zeros((128, TCH, 6), np.float32)
        eaE6L[:, :, 4:6] = 1.0
        eaE6H[:, :, 4:6] = 1.0

        def mk_onehots(dr, TC):
            stA = np.zeros((128, TC, 128), BF)
            ssA = np.zeros((128, TC, 128), BF)
            pp, cc = np.nonzero(dr >= 0)
            vv = dr[pp, cc].astype(np.int64)
            stA[pp, cc, vv] = 1
            ssA[vv, cc, pp] = 1
            return (np.ascontiguousarray(stA.reshape(128, TC * 128)),
                    np.ascontiguousarray(ssA.reshape(128, TC * 128)))

        for (bmask, idxA, drA, eaA, oA, base) in (
            (e_isL, idxL, dstrelL, eaE6L, oL, 0),
            (~e_isL, idxH, dstrelH, eaE6H, oH, SPLIT),
        ):
            es = np.nonzero(bmask)[0]
            rows = table_row[e_src[es]] - base
            order = np.lexsort((rows, ewin[es]))
            es, rows = es[order], rows[order]
            w_sorted = ewin[es]
            k = np.arange(len(es))
            wstart = np.searchsorted(w_sorted, np.arange(WIN))
            posn = k - wstart[w_sorted]
            chunk = oA[w_sorted] + posn // 128
            p = posn % 128
            idxA[chunk * 128 + p] = rows
            drA[p, chunk] = eslot[es].astype(np.float32)
            eaA[p, chunk, 0:4] = e_ea[es]

        stL, ssL = mk_onehots(dstrelL, TCL)
        stH, ssH = mk_onehots(dstrelH, TCH)

        n0 = r * NSHARD
        n_real = max(0, min(NSHARD, N - n0))
        posn_all = win_of * 128 + slot_of
        xT6 = np.zeros((6, NSHARD), np.float32)
        xT6[5, :] = 1.0
        batchc = np.full((128, WIN), -1.0, np.float32)
        if n_real > 0:
            ids = np.arange(n_real)
            xT6[0:5, posn_all[ids]] = x[n0 + ids].T
            batchc[posn_all[ids] % 128, posn_all[ids] // 128] = batch[n0 + ids]
        # batch one-hot per window [128, WIN*32]
        sgall = np.zeros((128, WIN, 32), np.float32)
        pp, ww = np.nonzero(batchc >= 0)
        sgall[pp, ww, batchc[pp, ww].astype(np.int64)] = 1.0

        per_core.append(dict(
            idxL=np.ascontiguousarray(_wrap16(idxL.astype(np.int16))),
            idxH=np.ascontiguousarray(_wrap16(idxH.astype(np.int16))),
            stL=stL, stH=stH, ssL=ssL, ssH=ssH,
            eaE6L=np.ascontiguousarray(eaE6L.reshape(128, TCL * 6)).astype(BF),
            eaE6H=np.ascontiguousarray(eaE6H.reshape(128, TCH * 6)).astype(BF),
            sgall=np.ascontiguousarray(sgall.reshape(128, WIN * 32)).astype(BF),
            xT6=xT6.astype(BF), batchc=batchc.astype(BF),
        ))
    meta = dict(cnL=tuple(int(v) for v in cnL), cnH=tuple(int(v) for v in cnH),
                oL=oL, oH=oH, TCL=TCL, TCH=TCH, NGL=NGL, NGH=NGH,
                table_row=table_row)
    return per_core, meta


def _fold_weights(inp):
    w = {}
    f8 = np.float64
    W_in = np.asarray(inp["W_in"], f8)
    b_in = np.asarray(inp["b_in"], f8)

    def weT_blk(We5):
        out = np.zeros((HID, 20), f8)
        for h in range(H):
            for a in range(5):
                out[h * D:(h + 1) * D, 5 * h + a] = We5[a, h * D:(h + 1) * D]
        return out

    def weblk24(We5):
        out = np.zeros((24, HID), f8)
        for h in range(H):
            for a in range(5):
                out[6 * h + a, h * D:(h + 1) * D] = We5[a, h * D:(h + 1) * D]
        return out

    for l in range(2):
        Wq = np.asarray(inp["Wq"][l], f8); bq = np.asarray(inp["bq"][l], f8)
        Wk = np.asarray(inp["Wk"][l], f8); bk = np.asarray(inp["bk"][l], f8)
        Wv = np.asarray(inp["Wv"][l], f8); bv = np.asarray(inp["bv"][l], f8)
        We = np.asarray(inp["We"][l], f8); be = np.asarray(inp["be"][l], f8)
        Wskip = np.asarray(inp["Wskip"][l], f8)
        bskip = np.asarray(inp["bskip"][l], f8)
        Wbeta = np.asarray(inp["Wbeta"][l], f8)[:, 0]
        P = Wbeta[:HID] + Wbeta[2 * HID:]
        Q = Wbeta[HID:2 * HID] - Wbeta[2 * HID:]
        We5 = np.concatenate([We, be[None, :]], 0)
        wqe = Wq @ weT_blk(We5)
        bqe = bq @ weT_blk(We5)
        Wqrb = np.concatenate([Wq, wqe, Wskip, -(Wskip @ Q)[:, None]], 1)
        bqrb = np.concatenate([bq, bqe, bskip, [-(bskip @ Q)]])
        Wkv = np.concatenate([Wk, Wv], 1)
        bkv = np.concatenate([bk, bv])
        if l == 0:
            w["wqrb0"] = np.concatenate(
                [W_in @ Wqrb, (b_in @ Wqrb + bqrb)[None, :]], 0).astype(BF)
            w["wkv0"] = np.concatenate(
                [W_in @ Wkv, (b_in @ Wkv + bkv)[None, :]], 0).astype(BF)
        else:
            w["wqrb1a"] = Wqrb[0:128].astype(BF)
            w["wqrb1b"] = np.concatenate([Wqrb[128:160], bqrb[None, :]], 0).astype(BF)
            w["wkv1a"] = Wkv[0:128].astype(BF)
            w["wkv1b"] = np.concatenate([Wkv[128:160], bkv[None, :]], 0).astype(BF)
        w[f"weblk{l}"] = weblk24(We5).astype(BF)
        # negated P replicated (so the beta pre-activation is nrbQ - outP)
        w[f"prepn{l}"] = np.broadcast_to((-P).astype(BF), (128, HID)).copy()

    Wg1 = np.asarray(inp["Wg1"], f8); bg1 = np.asarray(inp["bg1"], f8)
    w["wg1h1a"] = Wg1[0:128].astype(BF)
    w["wg1h1b"] = np.concatenate([Wg1[128:160], bg1[None, :]], 0).astype(BF)
    w["wg1h2a"] = Wg1[160:288].astype(BF)
    w["wg1h2b"] = Wg1[288:320].astype(BF)
    w["wg2rep"] = np.broadcast_to(
        np.asarray(inp["Wg2"], np.float32)[:, 0].astype(BF), (128, HID)).copy()
    w["bg2rep"] = np.full((128, 1), float(np.asarray(inp["bg2"]).reshape(-1)[0]),
                          np.float32)
    w["wh1"] = np.concatenate([np.asarray(inp["Wh1"], np.float32),
                               np.asarray(inp["bh1"], np.float32)[None, :]], 0)
    w["wh2"] = np.concatenate([np.asarray(inp["Wh2"], np.float32),
                               np.asarray(inp["bh2"], np.float32)[None, :]], 0)
    # constants
    w["identf"] = np.eye(128, dtype=np.float32)
    w["identb"] = np.eye(128, dtype=np.float32).astype(BF)
    w["epsT"] = np.full((128, 1), 1e-30, np.float32)
    w["oneT"] = np.ones((128, 1), np.float32)
    return w


# ============================ kernel build ============================

def _build(meta):
    cnL, cnH = meta["cnL"], meta["cnH"]
    oL, oH = meta["oL"], meta["oH"]
    TCL, TCH, NGL, NGH = meta["TCL"], meta["TCH"], meta["NGL"], meta["NGH"]
    CMAX = max(max(cnL), max(cnH))

    nc = bacc.Bacc(get_trn_type() or "TRN2", target_bir_lowering=False)

    d = {}
    def din(name, shape, dt):
        d[name] = nc.dram_tensor(name, shape, dt, kind="ExternalInput")
    din("xT6", [6, NSHARD], bf16)
    din("idxL", [128, NGL * GS // 16], i16)
    din("idxH", [128, NGH * GS // 16], i16)
    din("stL", [128, TCL * 128], bf16)
    din("stH", [128, TCH * 128], bf16)
    din("ssL", [128, TCL * 128], bf16)
    din("ssH", [128, TCH * 128], bf16)
    din("eaE6L", [128, TCL * 6], bf16)
    din("eaE6H", [128, TCH * 6], bf16)
    din("sgall", [128, WIN * 32], bf16)
    din("batchc", [128, WIN], bf16)
    for k, shp, dt in (
        ("wqrb0", [6, 341], bf16), ("wkv0", [6, 320], bf16),
        ("wqrb1a", [128, 341], bf16), ("wqrb1b", [33, 341], bf16),
        ("wkv1a", [128, 320], bf16), ("wkv1b", [33, 320], bf16),
        ("weblk0", [24, HID], bf16), ("weblk1", [24, HID], bf16),
        ("prepn0", [128, HID], bf16), ("prepn1", [128, HID], bf16),
        ("wg1h1a", [128, HID], bf16), ("wg1h1b", [33, HID], bf16),
        ("wg1h2a", [128, HID], bf16), ("wg1h2b", [32, HID], bf16),
        ("wg2rep", [128, HID], bf16), ("bg2rep", [128, 1], f32),
        ("wh1", [321, JK], f32), ("wh2", [321, 6], f32),
        ("identf", [128, 128], f32),
        ("identb", [128, 128], bf16),
        ("epsT", [128, 1], f32), ("oneT", [128, 1], f32),
    ):
        din(k, shp, dt)
    din("kvf0", [NPAD, ROW], bf16)
    out_d = nc.dram_tensor("out", [32, 6], f32, kind="ExternalOutput")

    kv_own1 = nc.dram_tensor("kv_own1", [NSHARD, ROW], bf16)
    kv_full1 = nc.dram_tensor("kv_full1", [NPAD, ROW], bf16, addr_space="Shared")
    pool_in = nc.dram_tensor("pool_in", [32, JK + 1], f32)
    pool_out = nc.dram_tensor("pool_out", [32, JK + 1], f32, addr_space="Shared")
    rg = [list(range(NCORES))]

    with tile.TileContext(nc) as tc:
        with (
            tc.tile_pool(name="const", bufs=1) as cst,
            tc.tile_pool(name="sb", bufs=2) as sb,
            tc.tile_pool(name="gath", bufs=4) as gath,
            tc.tile_pool(name="ps", bufs=2, space="PSUM") as ps,
        ):
            nc.gpsimd.load_library(mlp)
            regGS = nc.gpsimd.to_reg(GS)

            # ---- persistent constants ----
            C = {}
            for k in ("xT6", "eaE6L", "eaE6H", "sgall", "batchc",
                      "wqrb0", "wkv0", "wqrb1a", "wqrb1b",
                      "wkv1a", "wkv1b", "weblk0", "weblk1", "prepn0", "prepn1",
                      "wg1h1a", "wg1h1b", "wg1h2a", "wg1h2b", "wg2rep",
                      "bg2rep", "identf", "identb", "epsT", "oneT"):
                t = cst.tile(list(d[k].shape), d[k].dtype, name=f"c_{k}")
                nc.sync.dma_start(out=t[:], in_=d[k][:])
                C[k] = t
            for k, r0, nr in (("wh1a", 0, 128), ("wh1b", 128, 128),
                              ("wh1c", 256, 64), ("wh1d", 320, 1)):
                t = cst.tile([nr, JK], f32, name=f"c_{k}")
                nc.sync.dma_start(out=t[:], in_=d["wh1"][r0:r0 + nr, :])
                C[k] = t
            for k, r0, nr in (("wh2a", 0, 128), ("wh2b", 128, 128),
                              ("wh2c", 256, 64), ("wh2d", 320, 1)):
                t = cst.tile([nr, 6], f32, name=f"c_{k}")
                nc.sync.dma_start(out=t[:], in_=d["wh2"][r0:r0 + nr, :])
                C[k] = t
            idxLt = cst.tile([128, NGL * GS // 16], i16, name="idxLt")
            nc.sync.dma_start(out=idxLt[:], in_=d["idxL"][:])
            idxHt = cst.tile([128, NGH * GS // 16], i16, name="idxHt")
            nc.sync.dma_start(out=idxHt[:], in_=d["idxH"][:])
            ones1 = cst.tile([1, 128], f32, name="ones1")
            nc.gpsimd.memset(ones1[:], 1.0)

            # persistent h storage
            hTa = cst.tile([128, NSHARD], bf16, name="hTa")
            hTb = cst.tile([33, NSHARD], bf16, name="hTb")
            nc.gpsimd.memset(hTb[32:33, :], 1.0)
            hrows = [cst.tile([128, WIN * HID], bf16, name=f"hrows{l}")
                     for l in range(2)]

            # ---- edge phases (layer-0 kv table precomputed on host) ----
            for layer in range(2):
                wqrb_lhs = ((C["xT6"],) if layer == 0
                            else (hTa, hTb))
                wqrb_rhs = ((C["wqrb0"],) if layer == 0
                            else (C["wqrb1a"], C["wqrb1b"]))
                weblk = C[f"weblk{layer}"]
                prepn = C[f"prepn{layer}"]
                kvf = d["kvf0"] if layer == 0 else kv_full1

                with nc.named_scope(f"edge{layer}"):
                    cur = {"L": -1, "H": -1}
                    cur_tile = {"L": None, "H": None}

                    def _gather(region, gt):
                        if cur[region] == gt:
                            return cur_tile[region]
                        idxt, base, ng = ((idxLt, 0, NGL) if region == "L"
                                          else (idxHt, SPLIT, NGH))
                        gtile = gath.tile([128, GS // 128, ROW], bf16,
                                          tag="g" + region)
                        nc.gpsimd.dma_gather(
                            gtile[:], kvf[base : base + SPLIT, :],
                            idxt[:, gt * (GS // 16) : (gt + 1) * (GS // 16)],
                            num_idxs=GS, num_idxs_reg=regGS, elem_size=ROW)
                        cur[region] = gt
                        cur_tile[region] = gtile
                        return gtile

                    if layer == 1:
                        pgr = ps.tile([32, JK + 1], f32, tag="pgr", bufs=1)

                    for wdw in range(WIN):
                        wsl = slice(wdw * 128, (wdw + 1) * 128)
                        # window GEMM: qrb [128, 341] = q|qWe|r|nrbQ
                        qrb = ps.tile([128, 341], f32, tag="qrb", bufs=1)
                        for si, (lh, rh) in enumerate(zip(wqrb_lhs, wqrb_rhs)):
                            nc.tensor.matmul(qrb[:], lh[:, wsl], rh[:],
                                             start=(si == 0),
                                             stop=(si == len(wqrb_lhs) - 1))
                        qsb = sb.tile([128, 341], bf16, tag="qsb", bufs=2)
                        nc.scalar.activation(out=qsb[:], in_=qrb[:],
                                             func=AF.Copy)

                        # DMA the precomputed one-hot st/ssb slices
                        binfo = []
                        for (bk, o_, cn_) in (("L", oL, cnL), ("H", oH, cnH)):
                            cn = cn_[wdw]
                            c0 = int(o_[wdw])
                            stW = sb.tile([128, CMAX * 128], bf16,
                                          tag="stW" + bk, bufs=3)
                            nc.sync.dma_start(
                                out=stW[:, 0:cn * 128],
                                in_=d["st" + bk][:, c0 * 128:(c0 + cn) * 128])
                            ssbW = sb.tile([128, CMAX * 128], bf16,
                                           tag="ssbW" + bk, bufs=3)
                            nc.sync.dma_start(
                                out=ssbW[:, 0:cn * 128],
                                in_=d["ss" + bk][:, c0 * 128:(c0 + cn) * 128])
                            binfo.append((bk, c0, cn, stW, ssbW))

                        pacc = ps.tile([128, 184], f32, tag="acc", bufs=2)
                        nch = cnL[wdw] + cnH[wdw]
                        jglob = 0
                        for (bk, c0, cn, stW, ssbW) in binfo:
                            eaE = C["eaE6" + bk]
                            for j in range(cn):
                                c = c0 + j
                                gtile = _gather(bk, (c * 128) // GS)
                                sub = (c * 128 % GS) // 128
                                kv_g = gtile[:, sub, :]
                                jsl = slice(j * 128, (j + 1) * 128)
                                pqg = ps.tile([128, 180], f32, tag="pqg",
                                              bufs=3)
                                nc.tensor.matmul(pqg[:], ssbW[:, jsl],
                                                 qsb[:, 0:180],
                                                 start=True, stop=True)
                                tq = sb.tile([128, 180], bf16, tag="tq", bufs=4)
                                tqv = tq[:].rearrange("p (h s) -> p h s", h=H)
                                nc.vector.tensor_tensor(
                                    out=tqv[:, :, 0:40],
                                    in0=pqg[:, 0:160].rearrange(
                                        "p (h dd) -> p h dd", h=H),
                                    in1=kv_g[:, 0:160].rearrange(
                                        "p (h dd) -> p h dd", h=H),
                                    op=OP.mult)
                                nc.vector.tensor_tensor(
                                    out=tqv[:, :, 40:45],
                                    in0=pqg[:, 160:180].rearrange(
                                        "p (h a) -> p h a", h=H),
                                    in1=eaE[:, 6 * c:6 * c + 5].rearrange(
                                        "p (o a) -> p o a", o=1).to_broadcast(
                                        [128, H, 5]),
                                    op=OP.mult)
                                al = sb.tile([128, H], f32, tag="al", bufs=4)
                                nc.vector.tensor_reduce(
                                    out=al[:], in_=tqv,
                                    axis=mybir.AxisListType.X, op=OP.add)
                                ex = sb.tile([128, H], bf16, tag="ex", bufs=4)
                                nc.scalar.activation(out=ex[:], in_=al[:],
                                                     func=AF.Exp, scale=INVSQD)
                                wt = sb.tile([128, 184], bf16, tag="wt", bufs=4)
                                nc.vector.tensor_tensor(
                                    out=wt[:, 0:160].rearrange(
                                        "p (h dd) -> p h dd", h=H),
                                    in0=kv_g[:, 180:340].rearrange(
                                        "p (h dd) -> p h dd", h=H),
                                    in1=ex[:].rearrange(
                                        "p (h o) -> p h o", o=1).to_broadcast(
                                        [128, H, D]),
                                    op=OP.mult)
                                nc.vector.tensor_tensor(
                                    out=wt[:, 160:184].rearrange(
                                        "p (h a) -> p h a", h=H),
                                    in0=ex[:].rearrange(
                                        "p (h o) -> p h o", o=1).to_broadcast(
                                        [128, H, 6]),
                                    in1=eaE[:, 6 * c:6 * (c + 1)].rearrange(
                                        "p (o a) -> p o a", o=1).to_broadcast(
                                        [128, H, 6]),
                                    op=OP.mult)
                                nc.tensor.matmul(pacc[:], stW[:, jsl], wt[:],
                                                 start=(jglob == 0),
                                                 stop=(jglob == nch - 1),
                                                 skip_group_check=True)
                                jglob += 1

                        # ---- window tail ----
                        # B fixup: accumulate B24 @ weblk back into pacc's
                        # v-region (reopens the accumulation group)
                        b24 = sb.tile([128, 24], bf16, tag="b24", bufs=2)
                        nc.scalar.activation(out=b24[:], in_=pacc[:, 160:184],
                                             func=AF.Copy)
                        pbt = ps.tile([24, 128], bf16, tag="big", bufs=1)
                        nc.tensor.transpose(pbt[:], b24[:], C["identb"][:])
                        btb = sb.tile([24, 128], bf16, tag="btb", bufs=2)
                        nc.scalar.activation(out=btb[:], in_=pbt[:], func=AF.Copy)
                        nc.tensor.matmul(pacc[:, 0:160], btb[:], weblk[:],
                                         start=False, stop=True,
                                         skip_group_check=True)
                        accsb = sb.tile([128, HID], f32, tag="accsb", bufs=2)
                        nc.scalar.activation(out=accsb[:], in_=pacc[:, 0:160],
                                             func=AF.Copy)
                        # den + reciprocal (from b24; bf16 den is fine)
                        den = sb.tile([128, H], f32, tag="den", bufs=2)
                        nc.vector.tensor_tensor(
                            out=den[:].rearrange("p (h o) -> p h o", o=1),
                            in0=b24[:].rearrange(
                                "p (h s) -> p h s", h=H)[:, :, 5:6],
                            in1=C["epsT"][:, 0:1].rearrange(
                                "p (h o) -> p h o", h=1).to_broadcast(
                                [128, H, 1]),
                            op=OP.add)
                        denr = sb.tile([128, H], f32, tag="denr", bufs=2)
                        nc.vector.reciprocal(out=denr[:], in_=den[:])
                        outn = sb.tile([128, HID], bf16, tag="outn", bufs=2)
                        nc.vector.tensor_tensor(
                            out=outn[:].rearrange("p (h dd) -> p h dd", h=H),
                            in0=accsb[:].rearrange("p (h dd) -> p h dd", h=H),
                            in1=denr[:].rearrange(
                                "p (h o) -> p h o", o=1).to_broadcast(
                                [128, H, D]),
                            op=OP.mult)
                        # beta = 1/(1+exp(nrbQ - P.outn))
                        scr = sb.tile([128, HID], bf16, tag="scr", bufs=2)
                        nc.vector.tensor_tensor(out=scr[:], in0=outn[:],
                                                in1=prepn[:], op=OP.mult)
                        outPn = sb.tile([128, 1], f32, tag="outPn", bufs=2)
                        nc.vector.tensor_reduce(
                            out=outPn[:],
                            in_=scr[:].rearrange("p (a b) -> p a b", a=1),
                            axis=mybir.AxisListType.XY, op=OP.add)
                        ebt = sb.tile([128, 1], f32, tag="ebt", bufs=2)
                        nc.scalar.activation(out=ebt[:], in_=outPn[:],
                                             func=AF.Exp,
                                             bias=qsb[:, 340:341])
                        bp = sb.tile([128, 1], f32, tag="bp", bufs=2)
                        nc.vector.tensor_tensor(out=bp[:], in0=ebt[:],
                                                in1=C["oneT"][:], op=OP.add)
                        beta = sb.tile([128, 1], f32, tag="beta", bufs=2)
                        nc.vector.reciprocal(out=beta[:], in_=bp[:])
                        dvec = sb.tile([128, HID], bf16, tag="dvec", bufs=2)
                        nc.vector.tensor_tensor(out=dvec[:],
                                                in0=qsb[:, 180:340],
                                                in1=outn[:], op=OP.subtract)
                        hp = hrows[layer][:, wdw * HID:(wdw + 1) * HID]
                        nc.vector.scalar_tensor_tensor(
                            out=hp, in0=dvec[:], scalar=beta[:, 0:1],
                            in1=outn[:], op0=OP.mult, op1=OP.add)

                        if layer == 0:
                            # transpose h' into hTa/hTb + fused kv1 GEMM
                            ptr1 = ps.tile([128, 128], f32, tag="tp", bufs=2)
                            nc.tensor.transpose(ptr1[:], hp[:, 0:128],
                                                C["identb"][:])
                            nc.scalar.activation(out=hTa[:, wsl], in_=ptr1[:],
                                                 func=AF.Copy)
                            ptr2 = ps.tile([32, 128], f32, tag="tp2", bufs=2)
                            nc.tensor.transpose(ptr2[:], hp[:, 128:160],
                                                C["identb"][:])
                            nc.scalar.activation(out=hTb[0:32, wsl],
                                                 in_=ptr2[:], func=AF.Copy)
                            pkv = ps.tile([128, JK], f32, tag="big", bufs=1)
                            nc.tensor.matmul(pkv[:], hTa[:, wsl], C["wkv1a"][:],
                                             start=True, stop=False)
                            nc.tensor.matmul(pkv[:], hTb[:, wsl], C["wkv1b"][:],
                                             start=False, stop=True)
                            kvsb = sb.tile([128, 340], bf16, tag="kvsb", bufs=3)
                            nc.scalar.activation(out=kvsb[:, 0:HID],
                                                 in_=pkv[:, 0:HID],
                                                 func=AF.Copy)
                            nc.scalar.activation(out=kvsb[:, 180:340],
                                                 in_=pkv[:, HID:JK],
                                                 func=AF.Copy)
                            nc.sync.dma_start(out=kv_own1[wsl, 0:340],
                                              in_=kvsb[:])
                        else:
                            # fused gate + pool accumulation
                            ptr1 = ps.tile([128, 128], f32, tag="tp", bufs=2)
                            nc.tensor.transpose(ptr1[:], hp[:, 0:128],
                                                C["identb"][:])
                            t1 = sb.tile([128, 128], bf16, tag="t1", bufs=2)
                            nc.scalar.activation(out=t1[:], in_=ptr1[:],
                                                 func=AF.Copy)
                            ptr2 = ps.tile([32, 128], f32, tag="tp2", bufs=2)
                            nc.tensor.transpose(ptr2[:], hp[:, 128:160],
                                                C["identb"][:])
                            t2 = sb.tile([32, 128], bf16, tag="t2", bufs=2)
                            nc.scalar.activation(out=t2[:], in_=ptr2[:],
                                                 func=AF.Copy)
                            pg = ps.tile([128, HID], f32, tag="big", bufs=1)
                            nc.tensor.matmul(pg[:], hTa[:, wsl], C["wg1h1a"][:],
                                             start=True, stop=False)
                            nc.tensor.matmul(pg[:], hTb[:, wsl], C["wg1h1b"][:],
                                             start=False, stop=False)
                            nc.tensor.matmul(pg[:], t1[:], C["wg1h2a"][:],
                                             start=False, stop=False)
                            nc.tensor.matmul(pg[:], t2[:], C["wg1h2b"][:],
                                             start=False, stop=True)
                            grelu = sb.tile([128, HID], bf16, tag="grelu",
                                            bufs=2)
                            nc.scalar.activation(out=grelu[:], in_=pg[:],
                                                 func=AF.Relu)
                            scr2 = sb.tile([128, HID], bf16, tag="scr2", bufs=2)
                            nc.vector.tensor_tensor(out=scr2[:], in0=grelu[:],
                                                    in1=C["wg2rep"][:],
                                                    op=OP.mult)
                            gatec = sb.tile([128, 1], f32, tag="gatec", bufs=2)
                            nc.vector.tensor_reduce(
                                out=gatec[:],
                                in_=scr2[:].rearrange("p (a b) -> p a b", a=1),
                                axis=mybir.AxisListType.XY, op=OP.add)
                            ge = sb.tile([128, 1], f32, tag="ge", bufs=2)
                            nc.scalar.activation(out=ge[:], in_=gatec[:],
                                                 func=AF.Exp,
                                                 bias=C["bg2rep"][:, 0:1])
                            wg = sb.tile([128, JK + 1], bf16, tag="wg", bufs=2)
                            nc.vector.tensor_tensor(
                                out=wg[:, 0:HID],
                                in0=hrows[0][:, wdw * HID:(wdw + 1) * HID],
                                in1=ge[:, 0:1].to_broadcast([128, HID]),
                                op=OP.mult)
                            nc.vector.tensor_tensor(
                                out=wg[:, HID:JK], in0=hp,
                                in1=ge[:, 0:1].to_broadcast([128, HID]),
                                op=OP.mult)
                            nc.vector.tensor_copy(out=wg[:, JK:JK + 1],
                                                  in_=ge[:])
                            nc.tensor.matmul(
                                pgr[:],
                                C["sgall"][:, wdw * 32:(wdw + 1) * 32],
                                wg[:],
                                start=(wdw == 0),
                                stop=(wdw == WIN - 1),
                                skip_group_check=True)

                if layer == 0:
                    with nc.named_scope("ag1"):
                        nc.gpsimd.collective_compute(
                            "AllGather", OP.bypass, replica_groups=rg,
                            ins=[kv_own1[:]], outs=[kv_full1[:]])

            # ---- pool AllReduce + head MLP (fp32) ----
            with nc.named_scope("final"):
                pg_sb = sb.tile([32, JK + 1], f32, tag="pg_sb")
                nc.vector.tensor_copy(out=pg_sb[:], in_=pgr[:])
                nc.sync.dma_start(out=pool_in[:], in_=pg_sb[:])
                nc.gpsimd.collective_compute(
                    "AllReduce", OP.add, replica_groups=rg,
                    ins=[pool_in[:]], outs=[pool_out[:]])
                psb = sb.tile([32, JK + 1], f32, tag="psb")
                nc.sync.dma_start(out=psb[:], in_=pool_out[:])
                gden = sb.tile([32, 1], f32, tag="gden")
                nc.vector.tensor_tensor(out=gden[:], in0=psb[:, JK:JK + 1],
                                        in1=C["epsT"][0:32, :], op=OP.add)
                gdr = sb.tile([32, 1], f32, tag="gdr")
                nc.vector.reciprocal(out=gdr[:], in_=gden[:])
                pl = sb.tile([32, JK], f32, tag="pl")
                nc.vector.tensor_scalar_mul(pl[:], psb[:, 0:JK], gdr[:, 0:1])

                def _headmm(vin, wa, wb, wc, wd, nout, tagp):
                    pouts = ps.tile([32, nout], f32, tag=tagp, bufs=1)
                    for si, (c0, m) in enumerate(((0, 128), (128, 128),
                                                  (256, 64))):
                        ptt = ps.tile([m, 32], f32, tag="acc", bufs=2)
                        nc.tensor.transpose(ptt[:], vin[:, c0:c0 + m],
                                            C["identf"][0:32, 0:32])
                        tsb = sb.tile([m, 32], f32, tag="tsb", bufs=3)
                        nc.vector.tensor_copy(out=tsb[:], in_=ptt[:])
                        nc.tensor.matmul(pouts[:], tsb[:], (wa, wb, wc)[si][:m, :],
                                         start=(si == 0), stop=False,
                                         skip_group_check=True)
                    nc.tensor.matmul(pouts[:], ones1[:, :32], wd[:],
                                     start=False, stop=True,
                                     skip_group_check=True)
                    return pouts

                ph1 = _headmm(pl, C["wh1a"], C["wh1b"], C["wh1c"], C["wh1d"],
                              JK, "qrb")
                vrel = sb.tile([32, JK], f32, tag="vrel")
                nc.scalar.activation(out=vrel[:], in_=ph1[:], func=AF.Relu)
                ph2 = _headmm(vrel, C["wh2a"], C["wh2b"], C["wh2c"], C["wh2d"],
                              6, "big")
                osb = sb.tile([32, 6], f32, tag="osb")
                nc.vector.tensor_copy(out=osb[:], in_=ph2[:])
                nc.sync.dma_start(out=out_d[:], in_=osb[:])

    nc.compile()
    return nc


_CACHE = {}
_LAST_RES = None


def kernel(**inputs):
    inputs = {k: np.asarray(v) for k, v in inputs.items()}
    per_core, meta = _preprocess(
        inputs["x"], inputs["edge_index"], inputs["edge_attr"], inputs["batch"])
    w = _fold_weights(inputs)
    # host-side layer-0 kv table (= [x|1] @ wkv0), permuted to table rows
    x6 = np.concatenate([np.asarray(inputs["x"], np.float64),
                         np.ones((N, 1), np.float64)], 1)
    kvrows = x6 @ np.asarray(w["wkv0"], np.float64)
    kvf0 = np.zeros((NPAD, ROW), BF)
    kvf0[meta["table_row"][:N], 0:HID] = kvrows[:, 0:HID].astype(BF)
    kvf0[meta["table_row"][:N], 180:340] = kvrows[:, HID:JK].astype(BF)
    w = dict(w)
    w["kvf0"] = kvf0
    key = (meta["cnL"], meta["cnH"])
    if key not in _CACHE:
        _CACHE[key] = _build(meta)
    nc = _CACHE[key]
    in_maps = []
    for r in range(NCORES):
        m = dict(w)
        m.update(per_core[r])
        in_maps.append(m)
    import os
    trace = bool(os.environ.get("KERNEL_TRACE"))
    if trace:
        try:
            import axon_prof
            axon_prof.install()
        except Exception:
            trace = False
    res = run_bass_kernel_spmd(nc, in_maps, core_ids=list(range(NCORES)),
                               trace=trace)
    if trace and res.exec_time_ns is not None:
        print(f"HW exec time: {res.exec_time_ns} ns")
        if res.per_core_scope_times:
            for scope, cores in sorted(res.per_core_scope_times.items()):
                print(f"  scope {scope}: {cores}")
    global _LAST_RES
    _LAST_RES = res
    out = res.results[0]["out"]
    return out.reshape(G, 2, 3).astype(np.float32)
